# revision 16
# baseline (speedup 1.0000x reference)
"""Trainium2 Bass kernel: attention layer with KV cache, tensor-parallel over heads.

Sharding (8 NeuronCores): Megatron-style TP over the 32 heads -> 4 heads/core.
  - wq/wk/wv: column-parallel (each core owns a [512, 4096] output shard)
  - wo: row-parallel (each core owns wo[:, c*512:(c+1)*512]); cores emit
    partial o-proj outputs which the host sums (RowParallel unshard).
  - cache_k/cache_v: sharded along the head axis; history rows/positions are
    gathered host-side from batch_exec/start_pos (pure indexing).

v2 layout notes (vs the first working version):
  - Attention runs on 256-token sub-batches so score PSUM tiles are
    [128, 2, 256] = exactly one bank; QKV projections and o-proj keep
    512-wide moving operands.
  - 1/sqrt(hd) is folded into the exp's scale immediate; the router gate on
    new tokens is a per-partition bias AP (0 / -1e9) on the exp; the causal
    diagonal is one [128, 4, 128] triangular multiply per (sb, j).
  - softmax denominator comes from a DVE f32 chunk-tree + gpsimd
    partition_all_reduce instead of ones-matmuls, freeing the PE.
  - junk warm-up matmuls at t=0 keep the PE HAM clock-gate open while the
    first DMAs land.
"""

import numpy as np
import ml_dtypes

import concourse.bass as bass
import concourse.bacc as bacc
import concourse.tile as tile
from concourse import mybir
from concourse import bass_isa
from concourse.bass_utils import run_bass_kernel_spmd

BF16 = np.dtype(ml_dtypes.bfloat16)

# Problem shape (hardcoded per the task contract)
BSZ = 8
SEQ = 512
DIM = 4096
NH = 32
HD = 128
START = 512
KV = START + SEQ          # 1024
NC = 8                    # cores
HPC = NH // NC            # 4 heads per core
HF = HPC * HD             # 512 local features
P = 128
KC = DIM // P             # 32 contraction chunks
SB = 256                  # attention sub-batch (tokens)
NSB = SEQ // SB           # 2 sub-batches per batch
TH = START // P           # 4 history kv chunks
ROPE_BASE = 10000.0
SCALE = float(1.0 / np.sqrt(HD))
NWARM = 64

FP32 = mybir.dt.float32
BF16D = mybir.dt.bfloat16


def build_program():
    nc = bacc.Bacc(None, target_bir_lowering=False)
    x_d = nc.declare_dram_parameter("xp", [BSZ * NSB, P, KC, SB], BF16D, isOutput=False)
    wqk_d = nc.declare_dram_parameter("wqk", [2 * HPC, P, KC, P], BF16D, isOutput=False)
    wv_d = nc.declare_dram_parameter("wvp", [P, KC, HF], BF16D, isOutput=False)
    woT_d = nc.declare_dram_parameter("wop", [P, HPC, DIM], BF16D, isOutput=False)
    rope_d = nc.declare_dram_parameter("ropep", [BSZ, P, 2, SEQ], FP32, isOutput=False)
    kTh_d = nc.declare_dram_parameter("kThp", [BSZ, P, HPC, START], BF16D, isOutput=False)
    vh_d = nc.declare_dram_parameter("vhp", [BSZ, P, HPC * TH, HD], BF16D, isOutput=False)
    pen_d = nc.declare_dram_parameter("penp", [BSZ, P, HPC], FP32, isOutput=False)
    tri_d = nc.declare_dram_parameter("trip", [P, HPC, P], BF16D, isOutput=False)
    out_d = nc.declare_dram_parameter("out", [BSZ * SEQ, DIM], BF16D, isOutput=True)

    from contextlib import ExitStack

    with ExitStack() as ctx:
        tc = ctx.enter_context(tile.TileContext(nc))
        cpool = ctx.enter_context(tc.tile_pool(name="const", bufs=1))
        wupool = ctx.enter_context(tc.tile_pool(name="wu", bufs=2))
        xpool = ctx.enter_context(tc.tile_pool(name="xb", bufs=2))
        rpool = ctx.enter_context(tc.tile_pool(name="rope", bufs=1))
        qpool = ctx.enter_context(tc.tile_pool(name="qkv", bufs=2))
        hpool = ctx.enter_context(tc.tile_pool(name="hist", bufs=1))
        eepool = ctx.enter_context(tc.tile_pool(name="ee", bufs=2))
        wkpool = ctx.enter_context(tc.tile_pool(name="work", bufs=2))
        trpool = ctx.enter_context(tc.tile_pool(name="tree", bufs=2))
        smpool = ctx.enter_context(tc.tile_pool(name="small", bufs=2))
        apool = ctx.enter_context(tc.tile_pool(name="at", bufs=2))
        popool = ctx.enter_context(tc.tile_pool(name="po", bufs=2))
        pAcc = ctx.enter_context(tc.tile_pool(name="pAcc", bufs=2, space="PSUM"))
        pSc = ctx.enter_context(tc.tile_pool(name="pSc", bufs=4, space="PSUM"))
        pPv = ctx.enter_context(tc.tile_pool(name="pPv", bufs=2, space="PSUM"))

        # ---- warm-up: keep the PE HAM clock-gate open while DMAs land ----
        junk = cpool.tile([P, P], BF16D, tag="junk")
        nc.vector.memset(junk[:], 0.0)
        wps = pAcc.tile([P, HF], FP32, tag="acc")
        for _ in range(NWARM):
            nc.tensor.matmul(wps[0:1, 0:P], junk[:, 0:1], junk[:],
                             start=True, stop=True, skip_group_check=True)

        wv_s = cpool.tile([P, KC, HF], BF16D, tag="wv")
        woT_s = cpool.tile([P, HPC, DIM], BF16D, tag="wo")
        tri_s = cpool.tile([P, HPC, P], BF16D, tag="tri")

        def emit_oproj(aT, ts0):
            for sc2 in range(SB // P):
                for og in range(4):
                    pout = popool.tile([P, 2, HF], BF16D, tag="pout", name="pout")
                    for oi in range(2):
                        oc = og * 2 + oi
                        pp = pAcc.tile([P, HF], FP32, tag="acc", name="pp")
                        for h in range(HPC):
                            nc.tensor.matmul(
                                pp[:],
                                aT[:, h, sc2 * P:(sc2 + 1) * P],
                                woT_s[:, h, oc * HF:(oc + 1) * HF],
                                start=(h == 0), stop=(h == HPC - 1),
                            )
                        if oi == 0:
                            nc.vector.tensor_copy(pout[:, oi, :], pp[:])
                        else:
                            nc.scalar.activation(
                                pout[:, oi, :], pp[:],
                                mybir.ActivationFunctionType.Copy,
                            )
                    nc.sync.dma_start(
                        out_d[ts0 + sc2 * P: ts0 + (sc2 + 1) * P,
                              og * 2 * HF:(og * 2 + 2) * HF],
                        pout[:],
                    )

        pending = None
        for b in range(BSZ):
            # ---- per-batch DMAs ----
            xb = [xpool.tile([P, KC, SB], BF16D, tag="x", bufs=3, name=f"xb{sb_}")
                  for sb_ in range(NSB)]
            for sb in range(NSB):
                for xc in range(4):
                    nc.sync.dma_start(
                        xb[sb][:, xc * 8:(xc + 1) * 8, :],
                        x_d[b * NSB + sb, :, xc * 8:(xc + 1) * 8, :],
                    )
            rope_b = rpool.tile([P, 2, SEQ], FP32, tag="rope")
            nc.sync.dma_start(rope_b[:], rope_d[b])
            kThb = hpool.tile([P, HPC, START], BF16D, tag="kTh")
            nc.sync.dma_start(kThb[:], kTh_d[b])
            vhb = hpool.tile([P, HPC * TH, HD], BF16D, tag="vh")
            nc.sync.dma_start(vhb[:], vh_d[b])
            penb = smpool.tile([P, HPC], FP32, tag="pen", bufs=2)
            nc.sync.dma_start(penb[:], pen_d[b])

            # ---- phase A: QKV projections (+RoPE for q/k) ----
            qT = [qpool.tile([P, HPC, SB], BF16D, tag="qT", bufs=3, name=f"qT{sb_}")
                  for sb_ in range(NSB)]
            kT_b = qpool.tile([P, HPC, SEQ], BF16D, tag="kT")
            v_b = qpool.tile([P, HPC, HF], BF16D, tag="v")

            for proj in range(2):  # 0=q, 1=k
                if b == 0 and proj == 1:
                    # consts queue behind batch-0's critical startup DMAs
                    nc.sync.dma_start(wv_s[:, 0:16, :], wv_d[:, 0:16, :])
                    nc.sync.dma_start(wv_s[:, 16:32, :], wv_d[:, 16:32, :])
                    nc.sync.dma_start(woT_s[:, 0:2, :], woT_d[:, 0:2, :])
                    nc.sync.dma_start(woT_s[:, 2:4, :], woT_d[:, 2:4, :])
                    nc.sync.dma_start(tri_s[:], tri_d[:])
                for h in range(HPC):
                    wu0 = wupool.tile([P, KC // 2, P], BF16D, tag="wu")
                    wu1 = wupool.tile([P, KC // 2, P], BF16D, tag="wu")
                    nc.sync.dma_start(wu0[:], wqk_d[proj * HPC + h, :, 0:16, :])
                    nc.sync.dma_start(wu1[:], wqk_d[proj * HPC + h, :, 16:32, :])
                    for sb in range(NSB):
                        ps = pAcc.tile([P, SB], FP32, tag="acc")
                        for kc in range(KC):
                            wuc = wu0 if kc < 16 else wu1
                            nc.tensor.matmul(
                                ps[:], wuc[:, kc % 16, :], xb[sb][:, kc, :],
                                start=(kc == 0), stop=(kc == KC - 1),
                            )
                        # RoPE: dst = ps*cos + shift64(ps)*sin_signed
                        ck = rope_b[:, 0, sb * SB:(sb + 1) * SB]
                        sk = rope_b[:, 1, sb * SB:(sb + 1) * SB]
                        t1 = wkpool.tile([P, SB], FP32, tag="t1")
                        nc.vector.tensor_mul(t1[:], ps[:], ck)
                        t2 = wkpool.tile([P, SB], FP32, tag="t2")
                        H2 = HD // 2
                        nc.vector.tensor_mul(t2[0:H2, :], ps[H2:P, :], sk[0:H2, :])
                        nc.vector.tensor_mul(t2[H2:P, :], ps[0:H2, :], sk[H2:P, :])
                        if proj == 0:
                            dst = qT[sb][:, h, :]
                        else:
                            dst = kT_b[:, h, sb * SB:(sb + 1) * SB]
                        nc.vector.tensor_add(dst, t1[:], t2[:])

            for tc4 in range(HPC):  # v, natural [tok, feat] layout, 128-tok chunks
                sb = tc4 // 2
                ps = pAcc.tile([P, HF], FP32, tag="acc")
                for kc in range(KC):
                    nc.tensor.matmul(
                        ps[:], xb[sb][:, kc, (tc4 % 2) * P:(tc4 % 2 + 1) * P],
                        wv_s[:, kc, :],
                        start=(kc == 0), stop=(kc == KC - 1),
                    )
                nc.vector.tensor_copy(v_b[:, tc4, :], ps[:])

            # ---- phase B: attention per sub-batch ----
            for sb in range(NSB):
                nj = 2 * (sb + 1)          # visible new kv chunks
                eh = eepool.tile([P, HPC, TH, SB], BF16D, tag="eh")
                en = eepool.tile([P, HPC, HPC, SB], BF16D, tag="en")

                # history scores + exp (pair-merged, no gate: cache_mask==1)
                for h in range(HPC):
                    for pair in range(2):
                        sct = pSc.tile([P, 2, SB], FP32, tag="sc")
                        # one accumulation group per bank: on HW, start=True
                        # clears the whole PSUM bank, so disjoint chunks must
                        # share a group (start first / stop last).
                        for c in range(2):
                            t8 = pair * 2 + c
                            nc.tensor.matmul(
                                sct[:, c, :],
                                kThb[:, h, t8 * P:(t8 + 1) * P],
                                qT[sb][:, h, :],
                                start=(c == 0), stop=(c == 1),
                                skip_group_check=True,
                            )
                        nc.scalar.activation(
                            eh[:, h, pair * 2:pair * 2 + 2, :], sct[:],
                            mybir.ActivationFunctionType.Exp, scale=SCALE,
                        )
                # new-token scores + exp (router pen as bias; tri mask on diag)
                for j in range(nj):
                    c0 = 0 if j < nj - 1 else P
                    for hp in range(2):
                        sct = pSc.tile([P, 2, SB], FP32, tag="sc")
                        for c in range(2):
                            h = hp * 2 + c
                            nc.tensor.matmul(
                                sct[:, c, c0:],
                                kT_b[:, h, j * P:(j + 1) * P],
                                qT[sb][:, h, c0:],
                                start=(c == 0), stop=(c == 1),
                                skip_group_check=True,
                            )
                        nc.scalar.activation(
                            en[:, j, hp * 2:hp * 2 + 2, c0:], sct[:, :, c0:],
                            mybir.ActivationFunctionType.Exp, scale=SCALE,
                            bias=penb[:, j:j + 1],
                        )
                    if j >= nj - 2:  # diagonal block: causal triangle
                        d0 = (j - (nj - 2)) * P
                        nc.vector.tensor_mul(
                            en[:, j, :, d0:d0 + P], en[:, j, :, d0:d0 + P], tri_s[:]
                        )

                aT = apool.tile([P, HPC, SB], BF16D, tag="aT")
                for h in range(HPC):
                    # kv-sum: f32 chunk tree (DVE) + partition reduce (gpsimd)
                    rs = trpool.tile([P, SB], FP32, tag="rs")
                    nc.vector.tensor_add(rs[:], eh[:, h, 0, :], eh[:, h, 1, :])
                    nc.vector.tensor_add(rs[:], rs[:], eh[:, h, 2, :])
                    nc.vector.tensor_add(rs[:], rs[:], eh[:, h, 3, :])
                    for j in range(nj - 1):
                        nc.vector.tensor_add(rs[:], rs[:], en[:, j, h, :])
                    nc.vector.tensor_add(
                        rs[:, P:], rs[:, P:], en[:, nj - 1, h, P:]
                    )
                    rb = trpool.tile([P, SB], FP32, tag="rb")
                    nc.gpsimd.partition_all_reduce(
                        rb[:], rs[:], channels=P, reduce_op=bass_isa.ReduceOp.add
                    )
                    nc.vector.reciprocal_approx_fast(rb[:], rb[:])

                    # PV: oT[hd, s] accumulated over kv chunks
                    pot = pPv.tile([P, SB], FP32, tag="pv")
                    for t8 in range(TH):
                        nc.tensor.matmul(
                            pot[:], vhb[:, h * TH + t8, :], eh[:, h, t8, :],
                            start=(t8 == 0), stop=False, skip_group_check=True,
                        )
                    for j in range(nj):
                        c0 = 0 if j < nj - 1 else P
                        nc.tensor.matmul(
                            pot[:, c0:], v_b[:, j, h * P:(h + 1) * P],
                            en[:, j, h, c0:],
                            start=False, stop=(j == nj - 1), skip_group_check=True,
                        )
                    poS = smpool.tile([P, SB], FP32, tag="poS", bufs=3)
                    nc.scalar.activation(
                        poS[:], pot[:], mybir.ActivationFunctionType.Copy
                    )
                    nc.vector.tensor_mul(aT[:, h, :], poS[:], rb[:])

                # o-proj runs one sub-batch behind so the softmax-denominator
                # chain (DVE tree -> gpsimd reduce -> recip -> norm) finishes
                # in the shadow of the next sub-batch's PE work.
                if pending is not None:
                    emit_oproj(*pending)
                pending = (aT, b * SEQ + sb * SB)
        emit_oproj(*pending)
    nc.finalize()
    return nc


_CACHE = {}


def _get_program():
    if "nc" not in _CACHE:
        _CACHE["nc"] = build_program()
    return _CACHE["nc"]


def _prep_inputs(inputs):
    x = np.asarray(inputs["x"], np.float32)
    router = np.asarray(inputs["router"], np.float32)
    cache_k = np.asarray(inputs["cache_k"], np.float32)
    cache_v = np.asarray(inputs["cache_v"], np.float32)
    cache_mask = np.asarray(inputs["cache_mask"])
    wq = np.asarray(inputs["wq"], np.float32)
    wk = np.asarray(inputs["wk"], np.float32)
    wv = np.asarray(inputs["wv"], np.float32)
    wo = np.asarray(inputs["wo"], np.float32)
    position_ids = np.asarray(inputs["position_ids"], np.int64)
    batch_exec = np.asarray(inputs["batch_exec"], np.int64)
    start_pos = int(inputs["start_pos"])
    assert start_pos == START and x.shape == (BSZ, SEQ, DIM)
    # history gate must be all-ones for the biasless history-exp path
    assert bool(cache_mask[batch_exec, :START].all())

    # x packed per sub-batch into the SBUF tile layout [b*nsb, p, kc, tok]
    xT = x.reshape(BSZ, NSB, SB, KC, P)
    xp = np.ascontiguousarray(xT.transpose(0, 1, 4, 3, 2)).astype(BF16)

    # RoPE tables gathered at position_ids, packed [b, p(hd), {cos,sin}, tok]
    inv_freq = 1.0 / (ROPE_BASE ** (np.arange(0, HD, 2, dtype=np.float32) / HD))
    t = np.arange(KV, dtype=np.float32)
    emb = np.concatenate([t[:, None] * inv_freq, t[:, None] * inv_freq], axis=-1)
    cos_t = np.cos(emb).astype(np.float32)[position_ids]   # [8, 512, 128]
    sin_t = np.sin(emb).astype(np.float32)[position_ids]
    sign = np.where(np.arange(HD) < HD // 2, -1.0, 1.0).astype(np.float32)
    rope2 = np.stack([cos_t, sin_t * sign], axis=1)        # [8, 2, 512, 128]
    ropep = np.ascontiguousarray(rope2.transpose(0, 3, 1, 2)).astype(np.float32)

    # history cache slices (host-side gather = sharding)
    k_hist = cache_k[batch_exec, :, :START, :]   # [8, 32, 512, 128]
    v_hist = cache_v[batch_exec, :, :START, :]

    # router pen on new tokens as additive bias [b, p, j]
    pen_new = (router[:, :, 0] != 0.0)                     # [8, 512]
    penp = np.where(pen_new.reshape(BSZ, HPC, P), 0.0, -1e9)
    penp = np.ascontiguousarray(penp.transpose(0, 2, 1)).astype(np.float32)

    # upper-triangular diag-block mask, replicated over the 4 heads
    tri = np.triu(np.ones((P, P), np.float32))             # [p(kv), s]
    trip = np.ascontiguousarray(
        np.broadcast_to(tri[:, None, :], (P, HPC, P))).astype(BF16)

    in_maps = []
    for c in range(NC):
        hs, he = c * HPC, (c + 1) * HPC
        fs, fe = c * HF, (c + 1) * HF
        wqkT = np.stack([w[fs:fe].T for w in (wq, wk)])   # [2, 4096, 512]
        wqk = (wqkT.reshape(2, KC, P, HPC, HD).transpose(0, 3, 2, 1, 4)
               .reshape(2 * HPC, P, KC, HD))
        wvT = wv[fs:fe].T                                  # [4096, 512]
        wvp = wvT.reshape(KC, P, HF).transpose(1, 0, 2)    # [128, 32, 512]
        woTc = wo[:, fs:fe].T                              # [512, 4096]
        wop = woTc.reshape(HPC, P, DIM).transpose(1, 0, 2) # [128, 4, 4096]
        kThp = k_hist[:, hs:he].transpose(0, 3, 1, 2)      # [8, 128hd, 4h, 512]
        vhp = (v_hist[:, hs:he].reshape(BSZ, HPC, TH, P, HD)
               .transpose(0, 3, 1, 2, 4).reshape(BSZ, P, HPC * TH, HD))
        in_maps.append({
            "xp": xp.reshape(BSZ * NSB, P, KC, SB),
            "wqk": np.ascontiguousarray(wqk).astype(BF16),
            "wvp": np.ascontiguousarray(wvp).astype(BF16),
            "wop": np.ascontiguousarray(wop).astype(BF16),
            "ropep": ropep,
            "kThp": np.ascontiguousarray(kThp).astype(BF16),
            "vhp": np.ascontiguousarray(vhp).astype(BF16),
            "penp": penp,
            "trip": trip,
        })
    return in_maps


def _install_profile_hook():
    """The agent image's antenv lacks axon_hooks; shim it so trace=True works."""
    import sys, types
    if "antenv.axon_hooks" in sys.modules:
        return
    try:
        from trn_agent_boot.trn_boot import _ntff_profile_via_ctypes
    except ImportError:
        return
    mod = types.ModuleType("antenv.axon_hooks")
    mod._hook = _ntff_profile_via_ctypes("/opt/axon/libaxon_pjrt.so")

    def set_axon_ntff_profile_hook(h):
        mod._hook = h

    def get_axon_ntff_profile_hook():
        return mod._hook

    mod.set_axon_ntff_profile_hook = set_axon_ntff_profile_hook
    mod.get_axon_ntff_profile_hook = get_axon_ntff_profile_hook
    sys.modules["antenv.axon_hooks"] = mod
    import antenv
    antenv.axon_hooks = mod


def _run(inputs, trace=False):
    if trace:
        _install_profile_hook()
    nc = _get_program()
    in_maps = _prep_inputs(inputs)
    res = run_bass_kernel_spmd(nc, in_maps, core_ids=list(range(NC)), trace=trace)
    acc = np.zeros((BSZ * SEQ, DIM), np.float32)
    for c in range(NC):
        acc += res.results[c]["out"].astype(np.float32)
    return acc.reshape(BSZ, SEQ, DIM), res


def kernel(**inputs):
    out, _ = _run(inputs, trace=False)
    return out


# revision 20
# speedup vs baseline: 1.0312x; 1.0312x over previous
"""Trainium2 Bass kernel: attention layer with KV cache, tensor-parallel over heads.

Sharding (8 NeuronCores): Megatron-style TP over the 32 heads -> 4 heads/core.
  - wq/wk/wv: column-parallel (each core owns a [512, 4096] output shard)
  - wo: row-parallel (each core owns wo[:, c*512:(c+1)*512]); cores emit
    partial o-proj outputs which the host sums (RowParallel unshard).
  - cache_k/cache_v: sharded along the head axis; history rows/positions are
    gathered host-side from batch_exec/start_pos (pure indexing).

v2 layout notes (vs the first working version):
  - Attention runs on 256-token sub-batches so score PSUM tiles are
    [128, 2, 256] = exactly one bank; QKV projections and o-proj keep
    512-wide moving operands.
  - 1/sqrt(hd) is folded into the exp's scale immediate; the router gate on
    new tokens is a per-partition bias AP (0 / -1e9) on the exp; the causal
    diagonal is one [128, 4, 128] triangular multiply per (sb, j).
  - softmax denominator comes from a DVE f32 chunk-tree + gpsimd
    partition_all_reduce instead of ones-matmuls, freeing the PE.
  - junk warm-up matmuls at t=0 keep the PE HAM clock-gate open while the
    first DMAs land.
"""

import numpy as np
import ml_dtypes

import concourse.bass as bass
import concourse.bacc as bacc
import concourse.tile as tile
from concourse import mybir
from concourse import bass_isa
from concourse.bass_utils import run_bass_kernel_spmd

BF16 = np.dtype(ml_dtypes.bfloat16)

# Problem shape (hardcoded per the task contract)
BSZ = 8
SEQ = 512
DIM = 4096
NH = 32
HD = 128
START = 512
KV = START + SEQ          # 1024
NC = 8                    # cores
HPC = NH // NC            # 4 heads per core
HF = HPC * HD             # 512 local features
P = 128
KC = DIM // P             # 32 contraction chunks
SB = 256                  # attention sub-batch (tokens)
NSB = SEQ // SB           # 2 sub-batches per batch
TH = START // P           # 4 history kv chunks
ROPE_BASE = 10000.0
SCALE = float(1.0 / np.sqrt(HD))
NWARM = 64

FP32 = mybir.dt.float32
BF16D = mybir.dt.bfloat16


def build_program():
    nc = bacc.Bacc(None, target_bir_lowering=False)
    x_d = nc.declare_dram_parameter("xp", [BSZ * NSB, P, KC, SB], BF16D, isOutput=False)
    wqk_d = nc.declare_dram_parameter("wqk", [2 * HPC, P, KC, P], BF16D, isOutput=False)
    wv_d = nc.declare_dram_parameter("wvp", [P, KC, HF], BF16D, isOutput=False)
    woT_d = nc.declare_dram_parameter("wop", [P, HPC, DIM], BF16D, isOutput=False)
    rope_d = nc.declare_dram_parameter("ropep", [BSZ, P, 2, SEQ], FP32, isOutput=False)
    kTh_d = nc.declare_dram_parameter("kThp", [BSZ, P, HPC, START], BF16D, isOutput=False)
    vh_d = nc.declare_dram_parameter("vhp", [BSZ, P, HPC * TH, HD], BF16D, isOutput=False)
    pen_d = nc.declare_dram_parameter("penp", [BSZ, P, HPC], FP32, isOutput=False)
    tri_d = nc.declare_dram_parameter("trip", [P, HPC, P], BF16D, isOutput=False)
    out_d = nc.declare_dram_parameter("out", [BSZ * SEQ, DIM], BF16D, isOutput=True)

    from contextlib import ExitStack

    with ExitStack() as ctx:
        tc = ctx.enter_context(tile.TileContext(nc))
        cpool = ctx.enter_context(tc.tile_pool(name="const", bufs=1))
        wupool = ctx.enter_context(tc.tile_pool(name="wu", bufs=2))
        xpool = ctx.enter_context(tc.tile_pool(name="xb", bufs=2))
        rpool = ctx.enter_context(tc.tile_pool(name="rope", bufs=1))
        qpool = ctx.enter_context(tc.tile_pool(name="qkv", bufs=2))
        hpool = ctx.enter_context(tc.tile_pool(name="hist", bufs=1))
        eepool = ctx.enter_context(tc.tile_pool(name="ee", bufs=2))
        wkpool = ctx.enter_context(tc.tile_pool(name="work", bufs=2))
        smpool = ctx.enter_context(tc.tile_pool(name="small", bufs=2))
        apool = ctx.enter_context(tc.tile_pool(name="at", bufs=2))
        popool = ctx.enter_context(tc.tile_pool(name="po", bufs=2))
        pAcc = ctx.enter_context(tc.tile_pool(name="pAcc", bufs=3, space="PSUM"))
        pSc = ctx.enter_context(tc.tile_pool(name="pSc", bufs=3, space="PSUM"))
        pPv = ctx.enter_context(tc.tile_pool(name="pPv", bufs=2, space="PSUM"))

        # ---- warm-up: keep the PE HAM clock-gate open while DMAs land ----
        junk = cpool.tile([P, P], BF16D, tag="junk")
        nc.vector.memset(junk[:], 0.0)
        ones_bf = cpool.tile([P, 1], BF16D, tag="ones")
        nc.vector.memset(ones_bf[:], 1.0)
        wps = pAcc.tile([P, HF], FP32, tag="acc")
        for _ in range(NWARM):
            nc.tensor.matmul(wps[0:1, 0:P], junk[:, 0:1], junk[:],
                             start=True, stop=True, skip_group_check=True)

        wv_s = cpool.tile([P, KC, HF], BF16D, tag="wv")
        woT_s = cpool.tile([P, HPC, DIM], BF16D, tag="wo")
        tri_s = cpool.tile([P, HPC, P], BF16D, tag="tri")

        def emit_oproj(aT, ts0):
            for sc2 in range(SB // P):
                for og in range(4):
                    pout = popool.tile([P, 2, HF], BF16D, tag="pout", name="pout")
                    for oi in range(2):
                        oc = og * 2 + oi
                        pp = pAcc.tile([P, HF], FP32, tag="acc", name="pp")
                        for h in range(HPC):
                            nc.tensor.matmul(
                                pp[:],
                                aT[:, h, sc2 * P:(sc2 + 1) * P],
                                woT_s[:, h, oc * HF:(oc + 1) * HF],
                                start=(h == 0), stop=(h == HPC - 1),
                            )
                        nc.scalar.activation(
                            pout[:, oi, :], pp[:],
                            mybir.ActivationFunctionType.Copy,
                        )
                    nc.gpsimd.dma_start(
                        out_d[ts0 + sc2 * P: ts0 + (sc2 + 1) * P,
                              og * 2 * HF:(og * 2 + 2) * HF],
                        pout[:],
                    )

        pending = None
        for b in range(BSZ):
            # ---- per-batch DMAs ----
            xb = [xpool.tile([P, KC, SB], BF16D, tag="x", bufs=3, name=f"xb{sb_}")
                  for sb_ in range(NSB)]
            for sb in range(NSB):
                for xc in range(4):
                    nc.sync.dma_start(
                        xb[sb][:, xc * 8:(xc + 1) * 8, :],
                        x_d[b * NSB + sb, :, xc * 8:(xc + 1) * 8, :],
                    )
            rope_b = rpool.tile([P, 2, SEQ], FP32, tag="rope")
            nc.sync.dma_start(rope_b[:], rope_d[b])
            kThb = hpool.tile([P, HPC, START], BF16D, tag="kTh")
            nc.sync.dma_start(kThb[:], kTh_d[b])
            vhb = hpool.tile([P, HPC * TH, HD], BF16D, tag="vh")
            nc.sync.dma_start(vhb[:], vh_d[b])
            penb = smpool.tile([P, HPC], FP32, tag="pen", bufs=2)
            nc.sync.dma_start(penb[:], pen_d[b])

            # ---- phase A: QKV projections (+RoPE for q/k) ----
            qT = [qpool.tile([P, HPC, SB], BF16D, tag="qT", bufs=3, name=f"qT{sb_}")
                  for sb_ in range(NSB)]
            kT_b = qpool.tile([P, HPC, SEQ], BF16D, tag="kT")
            v_b = qpool.tile([P, HPC, HF], BF16D, tag="v")

            for proj in range(2):  # 0=q, 1=k
                if b == 0 and proj == 1:
                    # consts queue behind batch-0's critical startup DMAs
                    nc.sync.dma_start(wv_s[:, 0:16, :], wv_d[:, 0:16, :])
                    nc.sync.dma_start(wv_s[:, 16:32, :], wv_d[:, 16:32, :])
                    nc.sync.dma_start(woT_s[:, 0:2, :], woT_d[:, 0:2, :])
                    nc.sync.dma_start(woT_s[:, 2:4, :], woT_d[:, 2:4, :])
                    nc.sync.dma_start(tri_s[:], tri_d[:])
                for h in range(HPC):
                    wu0 = wupool.tile([P, KC // 2, P], BF16D, tag="wu")
                    wu1 = wupool.tile([P, KC // 2, P], BF16D, tag="wu")
                    nc.sync.dma_start(wu0[:], wqk_d[proj * HPC + h, :, 0:16, :])
                    nc.sync.dma_start(wu1[:], wqk_d[proj * HPC + h, :, 16:32, :])
                    for sb in range(NSB):
                        ps = pAcc.tile([P, SB], FP32, tag="acc")
                        for kc in range(KC):
                            wuc = wu0 if kc < 16 else wu1
                            nc.tensor.matmul(
                                ps[:], wuc[:, kc % 16, :], xb[sb][:, kc, :],
                                start=(kc == 0), stop=(kc == KC - 1),
                            )
                        # RoPE: dst = ps*cos + shift64(ps)*sin_signed
                        ck = rope_b[:, 0, sb * SB:(sb + 1) * SB]
                        sk = rope_b[:, 1, sb * SB:(sb + 1) * SB]
                        t1 = wkpool.tile([P, SB], FP32, tag="t1")
                        nc.vector.tensor_mul(t1[:], ps[:], ck)
                        t2 = wkpool.tile([P, SB], FP32, tag="t2")
                        H2 = HD // 2
                        nc.vector.tensor_mul(t2[0:H2, :], ps[H2:P, :], sk[0:H2, :])
                        nc.vector.tensor_mul(t2[H2:P, :], ps[0:H2, :], sk[H2:P, :])
                        if proj == 0:
                            dst = qT[sb][:, h, :]
                        else:
                            dst = kT_b[:, h, sb * SB:(sb + 1) * SB]
                        nc.vector.tensor_add(dst, t1[:], t2[:])

            for tc4 in range(HPC):  # v, natural [tok, feat] layout, 128-tok chunks
                sb = tc4 // 2
                ps = pAcc.tile([P, HF], FP32, tag="acc")
                for kc in range(KC):
                    nc.tensor.matmul(
                        ps[:], xb[sb][:, kc, (tc4 % 2) * P:(tc4 % 2 + 1) * P],
                        wv_s[:, kc, :],
                        start=(kc == 0), stop=(kc == KC - 1),
                    )
                nc.vector.tensor_copy(v_b[:, tc4, :], ps[:])

            # ---- phase B: attention per sub-batch ----
            for sb in range(NSB):
                nj = 2 * (sb + 1)          # visible new kv chunks
                eh = eepool.tile([P, HPC, TH, SB], BF16D, tag="eh")
                en = eepool.tile([P, HPC, HPC, SB], BF16D, tag="en")

                # history scores + exp (pair-merged, no gate: cache_mask==1)
                for h in range(HPC):
                    for pair in range(2):
                        sct = pSc.tile([P, 2, SB], FP32, tag="sc")
                        # one accumulation group per bank: on HW, start=True
                        # clears the whole PSUM bank, so disjoint chunks must
                        # share a group (start first / stop last).
                        for c in range(2):
                            t8 = pair * 2 + c
                            nc.tensor.matmul(
                                sct[:, c, :],
                                kThb[:, h, t8 * P:(t8 + 1) * P],
                                qT[sb][:, h, :],
                                start=(c == 0), stop=(c == 1),
                                skip_group_check=True,
                            )
                        nc.scalar.activation(
                            eh[:, h, pair * 2:pair * 2 + 2, :], sct[:],
                            mybir.ActivationFunctionType.Exp, scale=SCALE,
                        )
                # new-token scores + exp (router pen as bias; tri mask on diag)
                for j in range(nj):
                    c0 = 0 if j < nj - 1 else P
                    for hp in range(2):
                        sct = pSc.tile([P, 2, SB], FP32, tag="sc")
                        for c in range(2):
                            h = hp * 2 + c
                            nc.tensor.matmul(
                                sct[:, c, c0:],
                                kT_b[:, h, j * P:(j + 1) * P],
                                qT[sb][:, h, c0:],
                                start=(c == 0), stop=(c == 1),
                                skip_group_check=True,
                            )
                        nc.scalar.activation(
                            en[:, j, hp * 2:hp * 2 + 2, c0:], sct[:, :, c0:],
                            mybir.ActivationFunctionType.Exp, scale=SCALE,
                            bias=penb[:, j:j + 1],
                        )
                    if j >= nj - 2:  # diagonal block: causal triangle
                        d0 = (j - (nj - 2)) * P
                        nc.vector.tensor_mul(
                            en[:, j, :, d0:d0 + P], en[:, j, :, d0:d0 + P], tri_s[:]
                        )

                aT = apool.tile([P, HPC, SB], BF16D, tag="aT")
                for h in range(HPC):
                    # kv-sum: ones-matmul over the ee chunks (cheap on PE),
                    # then reciprocal + partition broadcast off the PE.
                    pr = pSc.tile([1, SB], FP32, tag="sc", name="pr")
                    for t8 in range(TH):
                        nc.tensor.matmul(
                            pr[:], ones_bf[:], eh[:, h, t8, :],
                            start=(t8 == 0), stop=False, skip_group_check=True,
                        )
                    for j in range(nj):
                        c0 = 0 if j < nj - 1 else P
                        nc.tensor.matmul(
                            pr[:, c0:], ones_bf[:], en[:, j, h, c0:],
                            start=False, stop=(j == nj - 1), skip_group_check=True,
                        )
                    rinv = smpool.tile([1, SB], FP32, tag="rinv", bufs=2)
                    nc.vector.reciprocal_approx_fast(rinv[:], pr[:])
                    rbb = smpool.tile([P, SB], FP32, tag="rbb", bufs=2)
                    nc.gpsimd.partition_broadcast(rbb[:], rinv[:])

                    # PV: oT[hd, s] accumulated over kv chunks
                    pot = pPv.tile([P, SB], FP32, tag="pv")
                    for t8 in range(TH):
                        nc.tensor.matmul(
                            pot[:], vhb[:, h * TH + t8, :], eh[:, h, t8, :],
                            start=(t8 == 0), stop=False, skip_group_check=True,
                        )
                    for j in range(nj):
                        c0 = 0 if j < nj - 1 else P
                        nc.tensor.matmul(
                            pot[:, c0:], v_b[:, j, h * P:(h + 1) * P],
                            en[:, j, h, c0:],
                            start=False, stop=(j == nj - 1), skip_group_check=True,
                        )
                    poS = smpool.tile([P, SB], FP32, tag="poS", bufs=3)
                    nc.scalar.activation(
                        poS[:], pot[:], mybir.ActivationFunctionType.Copy
                    )
                    nc.vector.tensor_mul(aT[:, h, :], poS[:], rbb[:])

                # o-proj runs one sub-batch behind so the softmax-denominator
                # chain (DVE tree -> gpsimd reduce -> recip -> norm) finishes
                # in the shadow of the next sub-batch's PE work.
                if pending is not None:
                    emit_oproj(*pending)
                pending = (aT, b * SEQ + sb * SB)
        emit_oproj(*pending)
    nc.finalize()
    return nc


_CACHE = {}


def _get_program():
    if "nc" not in _CACHE:
        _CACHE["nc"] = build_program()
    return _CACHE["nc"]


def _prep_inputs(inputs):
    x = np.asarray(inputs["x"], np.float32)
    router = np.asarray(inputs["router"], np.float32)
    cache_k = np.asarray(inputs["cache_k"], np.float32)
    cache_v = np.asarray(inputs["cache_v"], np.float32)
    cache_mask = np.asarray(inputs["cache_mask"])
    wq = np.asarray(inputs["wq"], np.float32)
    wk = np.asarray(inputs["wk"], np.float32)
    wv = np.asarray(inputs["wv"], np.float32)
    wo = np.asarray(inputs["wo"], np.float32)
    position_ids = np.asarray(inputs["position_ids"], np.int64)
    batch_exec = np.asarray(inputs["batch_exec"], np.int64)
    start_pos = int(inputs["start_pos"])
    assert start_pos == START and x.shape == (BSZ, SEQ, DIM)
    # history gate must be all-ones for the biasless history-exp path
    assert bool(cache_mask[batch_exec, :START].all())

    # x packed per sub-batch into the SBUF tile layout [b*nsb, p, kc, tok]
    xT = x.reshape(BSZ, NSB, SB, KC, P)
    xp = np.ascontiguousarray(xT.transpose(0, 1, 4, 3, 2)).astype(BF16)

    # RoPE tables gathered at position_ids, packed [b, p(hd), {cos,sin}, tok]
    inv_freq = 1.0 / (ROPE_BASE ** (np.arange(0, HD, 2, dtype=np.float32) / HD))
    t = np.arange(KV, dtype=np.float32)
    emb = np.concatenate([t[:, None] * inv_freq, t[:, None] * inv_freq], axis=-1)
    cos_t = np.cos(emb).astype(np.float32)[position_ids]   # [8, 512, 128]
    sin_t = np.sin(emb).astype(np.float32)[position_ids]
    sign = np.where(np.arange(HD) < HD // 2, -1.0, 1.0).astype(np.float32)
    rope2 = np.stack([cos_t, sin_t * sign], axis=1)        # [8, 2, 512, 128]
    ropep = np.ascontiguousarray(rope2.transpose(0, 3, 1, 2)).astype(np.float32)

    # history cache slices (host-side gather = sharding)
    k_hist = cache_k[batch_exec, :, :START, :]   # [8, 32, 512, 128]
    v_hist = cache_v[batch_exec, :, :START, :]

    # router pen on new tokens as additive bias [b, p, j]
    pen_new = (router[:, :, 0] != 0.0)                     # [8, 512]
    penp = np.where(pen_new.reshape(BSZ, HPC, P), 0.0, -1e9)
    penp = np.ascontiguousarray(penp.transpose(0, 2, 1)).astype(np.float32)

    # upper-triangular diag-block mask, replicated over the 4 heads
    tri = np.triu(np.ones((P, P), np.float32))             # [p(kv), s]
    trip = np.ascontiguousarray(
        np.broadcast_to(tri[:, None, :], (P, HPC, P))).astype(BF16)

    in_maps = []
    for c in range(NC):
        hs, he = c * HPC, (c + 1) * HPC
        fs, fe = c * HF, (c + 1) * HF
        wqkT = np.stack([w[fs:fe].T for w in (wq, wk)])   # [2, 4096, 512]
        wqk = (wqkT.reshape(2, KC, P, HPC, HD).transpose(0, 3, 2, 1, 4)
               .reshape(2 * HPC, P, KC, HD))
        wvT = wv[fs:fe].T                                  # [4096, 512]
        wvp = wvT.reshape(KC, P, HF).transpose(1, 0, 2)    # [128, 32, 512]
        woTc = wo[:, fs:fe].T                              # [512, 4096]
        wop = woTc.reshape(HPC, P, DIM).transpose(1, 0, 2) # [128, 4, 4096]
        kThp = k_hist[:, hs:he].transpose(0, 3, 1, 2)      # [8, 128hd, 4h, 512]
        vhp = (v_hist[:, hs:he].reshape(BSZ, HPC, TH, P, HD)
               .transpose(0, 3, 1, 2, 4).reshape(BSZ, P, HPC * TH, HD))
        in_maps.append({
            "xp": xp.reshape(BSZ * NSB, P, KC, SB),
            "wqk": np.ascontiguousarray(wqk).astype(BF16),
            "wvp": np.ascontiguousarray(wvp).astype(BF16),
            "wop": np.ascontiguousarray(wop).astype(BF16),
            "ropep": ropep,
            "kThp": np.ascontiguousarray(kThp).astype(BF16),
            "vhp": np.ascontiguousarray(vhp).astype(BF16),
            "penp": penp,
            "trip": trip,
        })
    return in_maps


def _install_profile_hook():
    """The agent image's antenv lacks axon_hooks; shim it so trace=True works."""
    import sys, types
    if "antenv.axon_hooks" in sys.modules:
        return
    try:
        from trn_agent_boot.trn_boot import _ntff_profile_via_ctypes
    except ImportError:
        return
    mod = types.ModuleType("antenv.axon_hooks")
    mod._hook = _ntff_profile_via_ctypes("/opt/axon/libaxon_pjrt.so")

    def set_axon_ntff_profile_hook(h):
        mod._hook = h

    def get_axon_ntff_profile_hook():
        return mod._hook

    mod.set_axon_ntff_profile_hook = set_axon_ntff_profile_hook
    mod.get_axon_ntff_profile_hook = get_axon_ntff_profile_hook
    sys.modules["antenv.axon_hooks"] = mod
    import antenv
    antenv.axon_hooks = mod


def _run(inputs, trace=False):
    if trace:
        _install_profile_hook()
    nc = _get_program()
    in_maps = _prep_inputs(inputs)
    res = run_bass_kernel_spmd(nc, in_maps, core_ids=list(range(NC)), trace=trace)
    acc = np.zeros((BSZ * SEQ, DIM), np.float32)
    for c in range(NC):
        acc += res.results[c]["out"].astype(np.float32)
    return acc.reshape(BSZ, SEQ, DIM), res


def kernel(**inputs):
    out, _ = _run(inputs, trace=False)
    return out
